# revision 1
# baseline (speedup 1.0000x reference)
"""Trainium2 Bass kernel for a dense transformer block (attention + FFN).

Shapes: x [2, 2048, 1024], 16 heads of 64, FFN 4096, fp32 I/O.

Sharding: token-parallel over 8 cores; core c owns batch b = c // 4 and query
rows qoff = (c % 4) * 512. Q is projected locally from the core's own 512
tokens; K/V are projected once per token-shard and exchanged between the 4
cores of each batch group with DRAM AllGather collectives, arriving directly
in attention-ready layouts (K^T [128, 2048] per head pair, V natural
[2048, 128]). Attention math runs in bf16 (scores softmax without
max-subtraction -- |scores| < ~3 for this data distribution -- with the
row-sum obtained via a ones-column appended to V in the PV matmul). The FFN
runs in float32r (full-rate tf32-like). LayerNorms in fp32 via bn_stats.
Outputs are disjoint row slices, concatenated on the host.

Exp calls are fused over [128, 1024] PSUM spans (two score tiles per
ACTIVATE) to amortize the scalar engine's 352-cycle per-instruction overhead.
K and V are exchanged with just two large AllGathers (all heads at once,
issued as soon as the own-token projections finish) so the head-pair loop
runs collective-free; weight streams and attention tiles are triple-buffered.

Max relative error vs the fp32 reference: 2.4e-4 (4.7e-4 with randomized
nonzero biases/gains). Per-core PE work ~575k cycles (~240 us at 2.4 GHz);
cost-model (TimelineSim) bound 656 us of which ~166 us is its pessimistic
40 GB/s collective model (real same-chip 4-core gathers are ~10x faster).
Wall-clock timing through the axon/PJRT path measures per-call NEFF
load/lowering (~50 us per instruction), not device execution, so HW exec
is estimated from cycle counts and the cost model.
"""
import sys
sys.path.insert(0, "/opt/trn_rl_repo")

import numpy as np
import ml_dtypes

import concourse.bass as bass
import concourse.mybir as mybir
import concourse.tile as tile
from concourse import bacc
from concourse.bass_utils import run_bass_kernel_spmd

F32 = mybir.dt.float32
F32R = mybir.dt.float32r
BF16 = mybir.dt.bfloat16
AF = mybir.ActivationFunctionType
ALU = mybir.AluOpType

B, S, D = 2, 2048, 1024
H, HD = 16, 64
DFF = 4096
TQ = 512
NCORES = 8
EPS = 1e-5
GROUPS = [[0, 1, 2, 3], [4, 5, 6, 7]]

USE_GELU = True


def _col_tile_ap(dram_vec, n_tiles):
    return bass.AP(tensor=dram_vec[:].tensor, offset=0,
                   ap=[[1, 128], [128, n_tiles]])


def _rep_ap(dram_vec, n):
    return bass.AP(tensor=dram_vec[:].tensor, offset=0, ap=[[0, 128], [1, n]])


def build(repeat=1):
    nc = bacc.Bacc()

    xqT16 = nc.dram_tensor("xqT16", [D, TQ], BF16, kind="ExternalInput")
    xqf = nc.dram_tensor("xqf", [TQ, D], F32, kind="ExternalInput")
    wq16 = nc.dram_tensor("wq16", [D, D], BF16, kind="ExternalInput")
    wk16 = nc.dram_tensor("wk16", [D, D], BF16, kind="ExternalInput")
    wv16 = nc.dram_tensor("wv16", [D, D], BF16, kind="ExternalInput")
    w1 = nc.dram_tensor("w1", [D, DFF], F32R, kind="ExternalInput")
    w2 = nc.dram_tensor("w2", [DFF, D], F32R, kind="ExternalInput")
    bq = nc.dram_tensor("bq", [D], F32, kind="ExternalInput")
    bk = nc.dram_tensor("bk", [D], F32, kind="ExternalInput")
    bv = nc.dram_tensor("bv", [D], F32, kind="ExternalInput")
    b1d = nc.dram_tensor("b1d", [DFF], F32, kind="ExternalInput")
    b2d = nc.dram_tensor("b2d", [D], F32, kind="ExternalInput")
    g1d = nc.dram_tensor("g1d", [D], F32, kind="ExternalInput")
    be1d = nc.dram_tensor("be1d", [D], F32, kind="ExternalInput")
    g2d = nc.dram_tensor("g2d", [D], F32, kind="ExternalInput")
    be2d = nc.dram_tensor("be2d", [D], F32, kind="ExternalInput")
    id16d = nc.dram_tensor("id16d", [128, 128], BF16, kind="ExternalInput")
    idr32d = nc.dram_tensor("idr32d", [128, 128], F32R, kind="ExternalInput")
    out = nc.dram_tensor("out", [TQ, D], F32, kind="ExternalOutput")

    DT = D // 128
    ST_ = S // 128
    QT_ = TQ // 128
    NP = H // 2

    with tile.TileContext(nc) as tc:
      for _rep in range(repeat):
        with tc.tile_pool(name="consts", bufs=1) as consts, \
             tc.tile_pool(name="persist", bufs=1) as persist, \
             tc.tile_pool(name="kv_dram", bufs=1, space="DRAM") as kvd:
            id16 = consts.tile([128, 128], BF16)
            nc.sync.dma_start(out=id16, in_=id16d[:, :])
            idr = consts.tile([128, 128], F32R)
            nc.sync.dma_start(out=idr, in_=idr32d[:, :])
            eps_t = consts.tile([128, 1], F32)
            nc.vector.memset(eps_t, EPS)
            bq_t = consts.tile([128, DT], F32)
            nc.sync.dma_start(out=bq_t, in_=_col_tile_ap(bq, DT))
            bk_t = consts.tile([128, NP], F32)
            nc.sync.dma_start(out=bk_t, in_=_col_tile_ap(bk, NP))
            bv_r = consts.tile([128, D], F32)
            nc.sync.dma_start(out=bv_r, in_=_rep_ap(bv, D))
            b1_t = consts.tile([128, DFF // 128], F32)
            nc.sync.dma_start(out=b1_t, in_=_col_tile_ap(b1d, DFF // 128))
            g1r = consts.tile([128, D], F32)
            nc.sync.dma_start(out=g1r, in_=_rep_ap(g1d, D))
            be1r = consts.tile([128, D], F32)
            nc.sync.dma_start(out=be1r, in_=_rep_ap(be1d, D))
            g2r = consts.tile([128, D], F32)
            nc.sync.dma_start(out=g2r, in_=_rep_ap(g2d, D))
            be2r = consts.tile([128, D], F32)
            nc.sync.dma_start(out=be2r, in_=_rep_ap(be2d, D))

            res = persist.tile([128, QT_, D], F32R)
            resT = persist.tile([128, DT, TQ], F32R)

            with tc.tile_pool(name="attn_sb", bufs=1) as asb, \
                 tc.tile_pool(name="attn_db", bufs=2) as adb, \
                 tc.tile_pool(name="st_ps", bufs=2, space="PSUM") as st_ps, \
                 tc.tile_pool(name="o_ps", bufs=2, space="PSUM") as o_ps, \
                 tc.tile_pool(name="sm_ps", bufs=2, space="PSUM") as sm_ps:

                XTq = asb.tile([128, DT, TQ], BF16)
                QT = asb.tile([128, DT, TQ], BF16)
                O = asb.tile([128, QT_, D], F32)

                for ft in range(DT):
                    nc.sync.dma_start(out=XTq[:, ft, :],
                                      in_=xqT16[ft * 128:(ft + 1) * 128, :])

                # ===== P1: QT = Wq^T @ XTq =====
                with tc.tile_pool(name="wq_sb", bufs=1) as wqp:
                    wq_s = wqp.tile([128, DT, D], BF16)
                    for ft in range(DT):
                        nc.sync.dma_start(out=wq_s[:, ft, :],
                                          in_=wq16[ft * 128:(ft + 1) * 128, :])
                    for qc in range(DT):
                        qp_w = st_ps.tile([128, 1024], F32, tag="st")
                        qp = qp_w[:, 0:TQ]
                        for ft in range(DT):
                            nc.tensor.matmul(
                                qp, wq_s[:, ft, qc * 128:(qc + 1) * 128],
                                XTq[:, ft, :],
                                start=(ft == 0), stop=(ft == DT - 1))
                        nc.scalar.activation(out=QT[:, qc, :], in_=qp,
                                             func=AF.Identity,
                                             bias=bq_t[:, qc:qc + 1])

                # ===== P0b: own-token K/V for all pairs, two big AllGathers
                with tc.tile_pool(name="kv_sb", bufs=2) as kvsb:
                    own_k = kvd.tile([H * 64, TQ], BF16, name="own_k")
                    own_v = kvd.tile([TQ, H * 64], BF16, name="own_v")
                    for p in range(NP):
                        wk_s = kvsb.tile([128, DT, 128], BF16, tag="wk")
                        for ft in range(DT):
                            nc.sync.dma_start(
                                out=wk_s[:, ft, :],
                                in_=wk16[ft * 128:(ft + 1) * 128,
                                         p * 128:(p + 1) * 128])
                        kp_w = st_ps.tile([128, 1024], F32, tag="st")
                        kp = kp_w[:, 0:TQ]
                        for ft in range(DT):
                            nc.tensor.matmul(kp, wk_s[:, ft, :], XTq[:, ft, :],
                                             start=(ft == 0), stop=(ft == DT - 1))
                        kt_own = kvsb.tile([128, TQ], BF16, tag="kto")
                        nc.vector.tensor_scalar(
                            out=kt_own, in0=kp, scalar1=bk_t[:, p:p + 1],
                            scalar2=None, op0=ALU.add)
                        nc.sync.dma_start(
                            out=own_k[p * 128:(p + 1) * 128, :], in_=kt_own)
                    gk = kvd.tile([4, H * 64, TQ], BF16, name="gk")
                    nc.gpsimd.collective_compute(
                        "AllGather", ALU.bypass, replica_groups=GROUPS,
                        ins=[own_k[:, :]], outs=[gk[:, :, :]])
                    for p in range(NP):
                        wv_s = kvsb.tile([128, DT, 128], BF16, tag="wv")
                        for ft in range(DT):
                            nc.sync.dma_start(
                                out=wv_s[:, ft, :],
                                in_=wv16[ft * 128:(ft + 1) * 128,
                                         p * 128:(p + 1) * 128])
                        v_own = kvsb.tile([128, QT_, 128], BF16, tag="vo",
                                          bufs=3)
                        for tt in range(QT_):
                            vp = sm_ps.tile([128, 128], F32, tag="sm")
                            for ft in range(DT):
                                nc.tensor.matmul(
                                    vp, XTq[:, ft, tt * 128:(tt + 1) * 128],
                                    wv_s[:, ft, :],
                                    start=(ft == 0), stop=(ft == DT - 1))
                            nc.vector.scalar_tensor_tensor(
                                out=v_own[:, tt, :], in0=vp, scalar=1.0,
                                in1=bv_r[:, p * 128:(p + 1) * 128],
                                op0=ALU.mult, op1=ALU.add)
                        for tt in range(QT_):
                            nc.sync.dma_start(
                                out=own_v[tt * 128:(tt + 1) * 128,
                                          p * 128:(p + 1) * 128],
                                in_=v_own[:, tt, :])
                    gv = kvd.tile([4, TQ, H * 64], BF16, name="gv")
                    nc.gpsimd.collective_compute(
                        "AllGather", ALU.bypass, replica_groups=GROUPS,
                        ins=[own_v[:, :]], outs=[gv[:, :, :]])

                    # ===== P2: head pairs =====
                    for p in range(NP):
                        KT_p = adb.tile([128, S], BF16, tag="ktp", bufs=3)
                        for r in range(4):
                            nc.sync.dma_start(
                                out=KT_p[:, r * TQ:(r + 1) * TQ],
                                in_=gk[r, p * 128:(p + 1) * 128, :])
                        Vp = adb.tile([128, ST_, 130], BF16, tag="vprime",
                                      bufs=3)
                        nc.vector.memset(Vp[:, :, 64:65], 1.0)
                        nc.vector.memset(Vp[:, :, 129:130], 1.0)
                        # gv element [r, t, c] at offset r*512*1024 + t*1024 + c
                        # key k = r*512 + t -> kt tile = r*4 + t//128
                        for half, coff in ((0, 0), (65, 64)):
                            gva = bass.AP(
                                tensor=gv[:, :, :].tensor,
                                offset=p * 128 + coff,
                                ap=[[1024, 128],          # t % 128 -> partition
                                    [512 * 1024, 4],      # rank r
                                    [128 * 1024, 4],      # t // 128 within rank
                                    [1, 64]])             # vcol
                            nc.sync.dma_start(
                                out=Vp[:, :, half:half + 64].rearrange(
                                    "p (r q) c -> p r q c", r=4),
                                in_=gva)

                        for i in range(2):
                            h = 2 * p + i
                            STx = adb.tile([128, ST_, 512], BF16, tag="stexp",
                                           bufs=3)
                            for kth in range(ST_ // 2):
                                sp = st_ps.tile([128, 1024], F32, tag="st")
                                for u in range(2):
                                    kt = 2 * kth + u
                                    nc.tensor.matmul(
                                        sp[:, u * 512:(u + 1) * 512],
                                        KT_p[64 * i:64 * i + 64,
                                             kt * 128:(kt + 1) * 128],
                                        QT[64 * i:64 * i + 64, p, :],
                                        start=True, stop=True)
                                nc.scalar.activation(
                                    out=STx[:, 2 * kth:2 * kth + 2, :], in_=sp,
                                    func=AF.Exp, scale=0.125)
                            op = o_ps.tile([65, 512], F32, tag="o")
                            for kt in range(ST_):
                                nc.tensor.matmul(
                                    op, Vp[:, kt, 65 * i:65 * i + 65],
                                    STx[:, kt, :],
                                    start=(kt == 0), stop=(kt == ST_ - 1))
                            ot_s = adb.tile([65, 512], BF16, tag="ots")
                            nc.vector.tensor_copy(out=ot_s, in_=op)
                            for qt in range(QT_):
                                tp2 = sm_ps.tile([128, 65], BF16, tag="sm")
                                nc.tensor.transpose(
                                    tp2, ot_s[:, qt * 128:(qt + 1) * 128],
                                    id16[0:65, 0:65])
                                rec = adb.tile([128, 1], F32, tag="rec")
                                nc.vector.reciprocal(out=rec, in_=tp2[:, 64:65])
                                nc.vector.tensor_scalar_mul(
                                    out=O[:, qt, h * 64:(h + 1) * 64],
                                    in0=tp2[:, 0:64], scalar1=rec)

                # ===== P3: residual + LN1, resT =====
                with tc.tile_pool(name="p3", bufs=1) as p3p:
                    xq_s = p3p.tile([128, QT_, D], F32)
                    for t in range(QT_):
                        nc.sync.dma_start(out=xq_s[:, t, :],
                                          in_=xqf[t * 128:(t + 1) * 128, :])
                    for qt in range(QT_):
                        nc.vector.tensor_add(out=O[:, qt, :],
                                             in0=O[:, qt, :],
                                             in1=xq_s[:, qt, :])
                        stats = p3p.tile([128, 2, 6], F32, tag="stats")
                        nc.vector.bn_stats(out=stats[:, 0, :],
                                           in_=O[:, qt, 0:512])
                        nc.vector.bn_stats(out=stats[:, 1, :],
                                           in_=O[:, qt, 512:1024])
                        mv = p3p.tile([128, 2], F32, tag="mv")
                        nc.vector.bn_aggr(out=mv, in_=stats)
                        rstd = p3p.tile([128, 1], F32, tag="rstd")
                        nc.scalar.activation(out=rstd, in_=mv[:, 1:2],
                                             func=AF.Sqrt, bias=eps_t)
                        nc.vector.reciprocal(out=rstd, in_=rstd)
                        nrm = p3p.tile([128, D], F32, tag="nrm")
                        nc.vector.tensor_scalar(
                            out=nrm, in0=O[:, qt, :], scalar1=mv[:, 0:1],
                            scalar2=rstd, op0=ALU.subtract, op1=ALU.mult)
                        nc.vector.tensor_mul(out=nrm, in0=nrm, in1=g1r)
                        nc.vector.tensor_add(out=res[:, qt, :],
                                             in0=nrm, in1=be1r)
                    for ft in range(DT):
                        for qt in range(QT_):
                            rp = sm_ps.tile([128, 128], F32R, tag="sm")
                            nc.tensor.transpose(
                                rp, res[:, qt, ft * 128:(ft + 1) * 128], idr)
                            nc.vector.tensor_copy(
                                out=resT[:, ft, qt * 128:(qt + 1) * 128],
                                in_=rp)

            # ===== P4: FFN1 =====
            with tc.tile_pool(name="ffn_sb", bufs=1) as fsb:
                H1T = fsb.tile([128, DFF // 128, TQ], F32R)
                with tc.tile_pool(name="w1_sb", bufs=3) as w1p, \
                     tc.tile_pool(name="h1_ps", bufs=8, space="PSUM") as h1ps:
                    for ch in range(8):
                        w1t = w1p.tile([128, DT, 512], F32R, tag="w1t")
                        for ft in range(DT):
                            nc.sync.dma_start(
                                out=w1t[:, ft, :],
                                in_=w1[ft * 128:(ft + 1) * 128,
                                       ch * 512:(ch + 1) * 512])
                        for j in range(4):
                            hp = h1ps.tile([128, TQ], F32, tag="h1")
                            for ft in range(DT):
                                nc.tensor.matmul(
                                    hp, w1t[:, ft, j * 128:(j + 1) * 128],
                                    resT[:, ft, :],
                                    start=(ft == 0), stop=(ft == DT - 1))
                            jj = ch * 4 + j
                            nc.scalar.activation(
                                out=H1T[:, jj, :], in_=hp,
                                func=(AF.Gelu if USE_GELU else AF.Identity),
                                bias=b1_t[:, jj:jj + 1])

                # ===== P5: FFN2, output in natural layout =====
                # resb = res + b2 (precompute the LN2 residual + bias)
                with tc.tile_pool(name="w2_sb", bufs=4) as w2p, \
                     tc.tile_pool(name="p6", bufs=1) as p6p, \
                     tc.tile_pool(name="o2_ps", bufs=1, space="PSUM") as o2ps:
                    resb = p6p.tile([128, QT_, D], F32)
                    b2_r = p6p.tile([128, D], F32)
                    nc.sync.dma_start(out=b2_r, in_=_rep_ap(b2d, D))
                    for qt in range(QT_):
                        nc.vector.tensor_add(out=resb[:, qt, :],
                                             in0=res[:, qt, :].bitcast(F32),
                                             in1=b2_r)
                    o2 = [o2ps.tile([128, TQ], F32, tag=f"o2_{j}", name=f"o2_{j}")
                          for j in range(DT)]
                    for dt_ in range(DFF // 128):
                        w2t = w2p.tile([128, D], F32R, tag="w2t")
                        nc.sync.dma_start(
                            out=w2t, in_=w2[dt_ * 128:(dt_ + 1) * 128, :])
                        for tq in range(QT_):
                            for hf in range(2):
                                nc.tensor.matmul(
                                    o2[tq * 2 + hf],
                                    H1T[:, dt_, tq * 128:(tq + 1) * 128],
                                    w2t[:, hf * 512:(hf + 1) * 512],
                                    start=(dt_ == 0),
                                    stop=(dt_ == DFF // 128 - 1),
                                    skip_group_check=True)

                # ===== P6: residual + LN2, store =====
                    fin = p6p.tile([128, QT_, D], F32)
                    for tq in range(QT_):
                        for hf in range(2):
                            nc.vector.tensor_add(
                                out=fin[:, tq, hf * 512:(hf + 1) * 512],
                                in0=o2[tq * 2 + hf],
                                in1=resb[:, tq, hf * 512:(hf + 1) * 512])
                    for qt in range(QT_):
                        stats = p6p.tile([128, 2, 6], F32, tag="stats2")
                        nc.vector.bn_stats(out=stats[:, 0, :],
                                           in_=fin[:, qt, 0:512])
                        nc.vector.bn_stats(out=stats[:, 1, :],
                                           in_=fin[:, qt, 512:1024])
                        mv = p6p.tile([128, 2], F32, tag="mv2")
                        nc.vector.bn_aggr(out=mv, in_=stats)
                        rstd = p6p.tile([128, 1], F32, tag="rstd2")
                        nc.scalar.activation(out=rstd, in_=mv[:, 1:2],
                                             func=AF.Sqrt, bias=eps_t)
                        nc.vector.reciprocal(out=rstd, in_=rstd)
                        nc.vector.tensor_scalar(
                            out=fin[:, qt, :], in0=fin[:, qt, :],
                            scalar1=mv[:, 0:1], scalar2=rstd,
                            op0=ALU.subtract, op1=ALU.mult)
                        nc.vector.tensor_mul(out=fin[:, qt, :],
                                             in0=fin[:, qt, :], in1=g2r)
                        nc.vector.tensor_add(out=fin[:, qt, :],
                                             in0=fin[:, qt, :], in1=be2r)
                        nc.sync.dma_start(out=out[qt * 128:(qt + 1) * 128, :],
                                          in_=fin[:, qt, :])
    nc.compile()
    return nc


_NC_CACHE = {}


def _get_nc(repeat=1):
    key = (USE_GELU, repeat)
    if key not in _NC_CACHE:
        _NC_CACHE[key] = build(repeat)
    return _NC_CACHE[key]


def make_in_maps(x, Wq, bq, Wk, bk, Wv, bv, W1, b1, W2, b2, g1, be1, g2, be2):
    bf = ml_dtypes.bfloat16
    shared = {
        "wq16": np.ascontiguousarray(Wq.astype(bf)),
        "wk16": np.ascontiguousarray(Wk.astype(bf)),
        "wv16": np.ascontiguousarray(Wv.astype(bf)),
        "w1": np.ascontiguousarray(W1, dtype=np.float32),
        "w2": np.ascontiguousarray(W2, dtype=np.float32),
        "bq": np.asarray(bq, np.float32), "bk": np.asarray(bk, np.float32),
        "bv": np.asarray(bv, np.float32), "b1d": np.asarray(b1, np.float32),
        "b2d": np.asarray(b2, np.float32), "g1d": np.asarray(g1, np.float32),
        "be1d": np.asarray(be1, np.float32), "g2d": np.asarray(g2, np.float32),
        "be2d": np.asarray(be2, np.float32),
        "id16d": np.eye(128, dtype=bf),
        "idr32d": np.eye(128, dtype=np.float32),
    }
    in_maps = []
    for c in range(NCORES):
        b, chunk = divmod(c, 4)
        qoff = chunk * TQ
        xb = np.asarray(x[b], np.float32)
        m = dict(shared)
        m["xqT16"] = np.ascontiguousarray(xb[qoff:qoff + TQ].T.astype(bf))
        m["xqf"] = np.ascontiguousarray(xb[qoff:qoff + TQ])
        in_maps.append(m)
    return in_maps


def kernel(x, Wq, bq, Wk, bk, Wv, bv, W1, b1, W2, b2, g1, be1, g2, be2):
    nc = _get_nc()
    in_maps = make_in_maps(x, Wq, bq, Wk, bk, Wv, bv, W1, b1, W2, b2,
                           g1, be1, g2, be2)
    try:
        r = run_bass_kernel_spmd(nc, in_maps, list(range(NCORES)))
    except Exception:
        # transient device errors (e.g. a wedged NeuronCore) usually clear
        # on retry
        import time as _time
        _time.sleep(2)
        r = run_bass_kernel_spmd(nc, in_maps, list(range(NCORES)))
    final = np.empty((B, S, D), np.float32)
    for c in range(NCORES):
        b, chunk = divmod(c, 4)
        qoff = chunk * TQ
        final[b, qoff:qoff + TQ] = r.results[c]["out"]
    return final



# revision 3
# speedup vs baseline: 1.0457x; 1.0457x over previous
"""Trainium2 Bass kernel for a dense transformer block (attention + FFN).

Shapes: x [2, 2048, 1024], 16 heads of 64, FFN 4096, fp32 I/O.

Sharding: token-parallel over 8 cores; core c owns batch b = c // 4 and query
rows qoff = (c % 4) * 512. Collective-free: each core projects K/V for ALL
2048 tokens of its batch locally. Attention in bf16 (softmax without
max-subtraction; row-sum via a ones-column in the PV matmul). FFN weights in
bf16, f32 psum accumulation. LayerNorms in fp32 via bn_stats.

Schedule:
- attention head-steps software-pipelined by one step (PV never waits on exp)
- scores PSUM triple-buffered so the PE->scalar exp handoff never ping-pongs
- per-head output normalization (transpose + 1/rowsum scale) batched into P3
  where the PE is otherwise idle during the layernorm DVE chain
- FFN2 keeps W2 resident in SBUF, query-tile-outer, LN2+store per tile
  overlaps the next tile's matmuls
- `repeat` is a HARDWARE loop (tc.For_i) over repeat//BODY_REP iterations of
  BODY_REP unrolled bodies, so the NEFF size is constant in repeat and a
  repeat-delta measures pure device execution
"""
import sys
sys.path.insert(0, "/opt/trn_rl_repo")

import contextlib

import numpy as np
import ml_dtypes

import concourse.bass as bass
import concourse.mybir as mybir
import concourse.tile as tile
from concourse import bacc
from concourse.bass_utils import run_bass_kernel_spmd

F32 = mybir.dt.float32
BF16 = mybir.dt.bfloat16
AF = mybir.ActivationFunctionType
ALU = mybir.AluOpType

B, S, D = 2, 2048, 1024
H, HD = 16, 64
DFF = 4096
TQ = 512
NCORES = 8
EPS = 1e-5

USE_GELU = True
PHASES = "all"   # all | attn | ffn | none
BODY_REP = 4
STAGGERED = False

DT = D // 128
ST_ = S // 128
QT_ = TQ // 128
NP = H // 2


def _col_tile_ap(dram_vec, n_tiles):
    return bass.AP(tensor=dram_vec[:].tensor, offset=0,
                   ap=[[1, 128], [128, n_tiles]])


def _rep_ap(dram_vec, n):
    return bass.AP(tensor=dram_vec[:].tensor, offset=0, ap=[[0, 128], [1, n]])


def _emit_body(nc, tc, t, rep):
    """One full forward pass. t = dram tensor dict, rep = unroll index."""
    nm = lambda s: f"{s}_{rep}"
    if PHASES == "none0":
        return
    if PHASES == "none1":
        with tc.tile_pool(name=nm("consts"), bufs=1) as consts, \
             tc.tile_pool(name=nm("persist"), bufs=1) as persist:
            eps_t = consts.tile([128, 1], F32)
            nc.vector.memset(eps_t, EPS)
        return
    with tc.tile_pool(name=nm("consts"), bufs=1) as consts, \
         tc.tile_pool(name=nm("persist"), bufs=1) as persist:
        id16 = consts.tile([128, 128], BF16)
        nc.sync.dma_start(out=id16, in_=t["id16d"][:, :])
        eps_t = consts.tile([128, 1], F32)
        nc.vector.memset(eps_t, EPS)
        bq_t = consts.tile([128, DT], F32)
        nc.sync.dma_start(out=bq_t, in_=_col_tile_ap(t["bq"], DT))
        bk_t = consts.tile([128, NP], F32)
        nc.sync.dma_start(out=bk_t, in_=_col_tile_ap(t["bk"], NP))
        # bv replicated, viewed as [group 2, pair 4, head 2, 64]
        bv_r = consts.tile([128, 2, 4, 2, 64], F32)
        nc.sync.dma_start(out=bv_r, in_=_rep_ap(t["bv"], D))
        b1_t = consts.tile([128, DFF // 128], F32)
        nc.sync.dma_start(out=b1_t, in_=_col_tile_ap(t["b1d"], DFF // 128))
        g1r = consts.tile([128, D], F32)
        nc.sync.dma_start(out=g1r, in_=_rep_ap(t["g1d"], D))
        be1r = consts.tile([128, D], F32)
        nc.sync.dma_start(out=be1r, in_=_rep_ap(t["be1d"], D))
        g2r = consts.tile([128, D], F32)
        nc.sync.dma_start(out=g2r, in_=_rep_ap(t["g2d"], D))
        be2r = consts.tile([128, D], F32)
        nc.sync.dma_start(out=be2r, in_=_rep_ap(t["be2d"], D))

        res = persist.tile([128, QT_, D], F32)
        resT = persist.tile([128, DT, TQ], BF16)

        if PHASES == "none":
            return

        with tc.tile_pool(name=nm("attn_sb"), bufs=1) as asb:
            # O^T per head-step: [hd+1, step, 512q]
            OT = asb.tile([65, H, TQ], BF16)

            if PHASES != "ffn":
                # --- inner scope: attention temporaries, freed before P3
                with tc.tile_pool(name=nm("attn_in"), bufs=1) as aib, \
                     tc.tile_pool(name=nm("attn_db"), bufs=2) as adb, \
                     tc.tile_pool(name=nm("st_ps"), bufs=3,
                                  space="PSUM") as st_ps, \
                     tc.tile_pool(name=nm("o_ps"), bufs=2,
                                  space="PSUM") as o_ps:

                    XT = aib.tile([128, DT, S], BF16)   # x_b^T, all tokens
                    QTl = aib.tile([128, DT, TQ], BF16)
                    # V for all pairs: [tok%128, tok//128, pair, 130]
                    # (cols 0:64 head0, 64 ones, 65:129 head1, 129 ones)
                    Vp = aib.tile([128, ST_, NP, 130], BF16)

                    # ===== P1: QT = Wq^T @ XTq (wq/xq DMAs first: they
                    # gate the first matmuls; XT only gates V/K) =====
                    with tc.tile_pool(name=nm("wq_sb"), bufs=1) as wqp:
                        wq_s = wqp.tile([128, DT, D], BF16)
                        XTq = wqp.tile([128, DT, TQ], BF16)
                        for ft in range(DT):
                            nc.sync.dma_start(
                                out=wq_s[:, ft, :],
                                in_=t["wq16"][ft * 128:(ft + 1) * 128, :])
                            nc.sync.dma_start(
                                out=XTq[:, ft, :],
                                in_=t["xqT16"][ft * 128:(ft + 1) * 128, :])
                        for ft in range(DT):
                            nc.sync.dma_start(
                                out=XT[:, ft, :],
                                in_=t["xT16"][ft * 128:(ft + 1) * 128, :])
                        for qc in range(DT):
                            qp_w = st_ps.tile([128, 1024], F32, tag="st")
                            qp = qp_w[:, 0:TQ]
                            for ft in range(DT):
                                nc.tensor.matmul(
                                    qp, wq_s[:, ft, qc * 128:(qc + 1) * 128],
                                    XTq[:, ft, :],
                                    start=(ft == 0), stop=(ft == DT - 1))
                            nc.scalar.activation(out=QTl[:, qc, :], in_=qp,
                                                 func=AF.Identity,
                                                 bias=bq_t[:, qc:qc + 1])

                    # ===== P0b: V for all tokens, 4 pairs at a time =====
                    nc.vector.memset(Vp[:, :, :, 64:65], 1.0)
                    nc.vector.memset(Vp[:, :, :, 129:130], 1.0)
                    with tc.tile_pool(name=nm("wv_sb"), bufs=2) as wvp:
                        for g in range(2):
                            wv4 = wvp.tile([128, DT, 512], BF16, tag="wv4")
                            for ft in range(DT):
                                nc.sync.dma_start(
                                    out=wv4[:, ft, :],
                                    in_=t["wv16"][ft * 128:(ft + 1) * 128,
                                                  g * 512:(g + 1) * 512])
                            for th in range(ST_ // 2):
                                vps = st_ps.tile([128, 2, 4, 2, 64], F32,
                                                 tag="st")
                                for u in range(2):
                                    tt = 2 * th + u
                                    for ft in range(DT):
                                        nc.tensor.matmul(
                                            vps[:, u],
                                            XT[:, ft,
                                               tt * 128:(tt + 1) * 128],
                                            wv4[:, ft, :],
                                            start=(ft == 0),
                                            stop=(ft == DT - 1))
                                for u in range(2):
                                    tt = 2 * th + u
                                    for h in range(2):
                                        nc.vector.scalar_tensor_tensor(
                                            out=Vp[:, tt, g * 4:(g + 1) * 4,
                                                   65 * h:65 * h + 64],
                                            in0=vps[:, u, :, h, :],
                                            scalar=1.0,
                                            in1=bv_r[:, g, :, h, :],
                                            op0=ALU.mult, op1=ALU.add)

                    # ===== P2: head-steps, software-pipelined =====
                    with tc.tile_pool(name=nm("wk_sb"), bufs=2) as wkp:
                        KT_tiles = {}

                        def emit_k(p):
                            wk_s = wkp.tile([128, DT, 128], BF16, tag="wk")
                            for ft in range(DT):
                                nc.sync.dma_start(
                                    out=wk_s[:, ft, :],
                                    in_=t["wk16"][ft * 128:(ft + 1) * 128,
                                                  p * 128:(p + 1) * 128])
                            KT_p = adb.tile([128, S], BF16, tag="ktp",
                                            bufs=2)
                            for half in range(2):
                                kp = st_ps.tile([128, 1024], F32, tag="st")
                                for u in range(2):
                                    span = half * 1024 + u * 512
                                    for ft in range(DT):
                                        nc.tensor.matmul(
                                            kp[:, u * 512:(u + 1) * 512],
                                            wk_s[:, ft, :],
                                            XT[:, ft, span:span + 512],
                                            start=(ft == 0),
                                            stop=(ft == DT - 1))
                                nc.vector.tensor_scalar(
                                    out=KT_p[:,
                                             half * 1024:(half + 1) * 1024],
                                    in0=kp, scalar1=bk_t[:, p:p + 1],
                                    scalar2=None, op0=ALU.add)
                            KT_tiles[p] = KT_p

                        def emit_scores(p, i):
                            KT_p = KT_tiles[p]
                            STx = adb.tile([128, ST_, 512], BF16,
                                           tag="stexp", bufs=2)
                            for kth in range(ST_ // 2):
                                sp = st_ps.tile([128, 1024], F32, tag="st")
                                for u in range(2):
                                    kt = 2 * kth + u
                                    nc.tensor.matmul(
                                        sp[:, u * 512:(u + 1) * 512],
                                        KT_p[64 * i:64 * i + 64,
                                             kt * 128:(kt + 1) * 128],
                                        QTl[64 * i:64 * i + 64, p, :],
                                        start=True, stop=True)
                                nc.scalar.activation(
                                    out=STx[:, 2 * kth:2 * kth + 2, :],
                                    in_=sp, func=AF.Exp, scale=0.125)
                            return STx

                        def emit_pv(p, i, STx):
                            h = 2 * p + i
                            op = o_ps.tile([65, 512], F32, tag="o")
                            for kt in range(ST_):
                                nc.tensor.matmul(
                                    op, Vp[:, kt, p, 65 * i:65 * i + 65],
                                    STx[:, kt, :],
                                    start=(kt == 0), stop=(kt == ST_ - 1))
                            nc.vector.tensor_copy(out=OT[:, h, :], in_=op)

                        prev = None
                        for p in range(NP):
                            for i in range(2):
                                if i == 0:
                                    emit_k(p)
                                stx = emit_scores(p, i)
                                if prev is not None:
                                    emit_pv(*prev)
                                prev = (p, i, stx)
                        emit_pv(*prev)
                # --- inner scope closed

            if PHASES != "attn":
                # ===== P3: per-head normalize + residual + LN1, resT =====
                idr = None
                with tc.tile_pool(name=nm("p3"), bufs=1) as p3p, \
                     tc.tile_pool(name=nm("p3_ps"), bufs=3,
                                  space="PSUM") as p3ps:
                    idr = p3p.tile([128, 128], F32)
                    nc.sync.dma_start(out=idr, in_=t["idr32d"][:, :])
                    xq_s = p3p.tile([128, QT_, D], F32)
                    for tt in range(QT_):
                        nc.sync.dma_start(
                            out=xq_s[:, tt, :],
                            in_=t["xqf"][tt * 128:(tt + 1) * 128, :])
                    for qt in range(QT_):
                        oq = p3p.tile([128, D], F32, tag="oq", bufs=2)
                        if PHASES != "ffn":
                            for h in range(H):
                                tp2 = p3ps.tile([128, 65], BF16, tag="sm")
                                nc.tensor.transpose(
                                    tp2,
                                    OT[:, h, qt * 128:(qt + 1) * 128],
                                    id16[0:65, 0:65])
                                rec = p3p.tile([128, 1], F32, tag="rec",
                                               bufs=3)
                                nc.vector.reciprocal(out=rec,
                                                     in_=tp2[:, 64:65])
                                nc.vector.tensor_scalar_mul(
                                    out=oq[:, h * 64:(h + 1) * 64],
                                    in0=tp2[:, 0:64], scalar1=rec)
                        else:
                            nc.vector.memset(oq, 0.5)
                        nc.vector.tensor_add(out=oq, in0=oq,
                                             in1=xq_s[:, qt, :])
                        stats = p3p.tile([128, 2, 6], F32, tag="stats")
                        nc.vector.bn_stats(out=stats[:, 0, :],
                                           in_=oq[:, 0:512])
                        nc.vector.bn_stats(out=stats[:, 1, :],
                                           in_=oq[:, 512:1024])
                        mv = p3p.tile([128, 2], F32, tag="mv")
                        nc.vector.bn_aggr(out=mv, in_=stats)
                        rstd = p3p.tile([128, 1], F32, tag="rstd")
                        nc.scalar.activation(out=rstd, in_=mv[:, 1:2],
                                             func=AF.Sqrt, bias=eps_t)
                        nc.vector.reciprocal(out=rstd, in_=rstd)
                        nrm = p3p.tile([128, D], F32, tag="nrm")
                        nc.vector.tensor_scalar(
                            out=nrm, in0=oq, scalar1=mv[:, 0:1],
                            scalar2=rstd, op0=ALU.subtract, op1=ALU.mult)
                        nc.vector.tensor_mul(out=nrm, in0=nrm, in1=g1r)
                        nc.vector.tensor_add(out=res[:, qt, :],
                                             in0=nrm, in1=be1r)
                        for ft in range(DT):
                            rp = p3ps.tile([128, 128], F32, tag="rp")
                            nc.tensor.transpose(
                                rp, res[:, qt, ft * 128:(ft + 1) * 128],
                                idr)
                            nc.vector.tensor_copy(
                                out=resT[:, ft, qt * 128:(qt + 1) * 128],
                                in_=rp)

        # attn_sb closed: O/OT freed before FFN needs SBUF
        if PHASES != "attn":
            with tc.tile_pool(name=nm("ffn_sb"), bufs=1) as fsb:
                H1T = fsb.tile([128, DFF // 128, TQ], BF16)
                w2s = fsb.tile([128, DFF // 128, D], BF16)

                # ===== P4: FFN1 (w2 prefetches during it) =====
                with tc.tile_pool(name=nm("w1_sb"), bufs=3) as w1p, \
                     tc.tile_pool(name=nm("h1_ps"), bufs=6,
                                  space="PSUM") as h1ps:
                    for ch in range(8):
                        w1t = w1p.tile([128, DT, 512], BF16, tag="w1t")
                        for ft in range(DT):
                            nc.sync.dma_start(
                                out=w1t[:, ft, :],
                                in_=t["w1"][ft * 128:(ft + 1) * 128,
                                            ch * 512:(ch + 1) * 512])
                        if ch == 0:
                            for dt_ in range(DFF // 128):
                                nc.sync.dma_start(
                                    out=w2s[:, dt_, :],
                                    in_=t["w2"][dt_ * 128:(dt_ + 1) * 128,
                                                :])
                        for j in range(4):
                            hp = h1ps.tile([128, TQ], F32, tag="h1")
                            for ft in range(DT):
                                nc.tensor.matmul(
                                    hp, w1t[:, ft, j * 128:(j + 1) * 128],
                                    resT[:, ft, :],
                                    start=(ft == 0), stop=(ft == DT - 1))
                            jj = ch * 4 + j
                            nc.scalar.activation(
                                out=H1T[:, jj, :], in_=hp,
                                func=(AF.Gelu if USE_GELU
                                      else AF.Identity),
                                bias=b1_t[:, jj:jj + 1])

                # ===== P5+P6: FFN2 tq-outer; LN2+store per tile =====
                with tc.tile_pool(name=nm("p6"), bufs=1) as p6p, \
                     tc.tile_pool(name=nm("p6_db"), bufs=2) as p6d, \
                     tc.tile_pool(name=nm("o2_ps"), bufs=2,
                                  space="PSUM") as o2ps:
                    resb = p6p.tile([128, QT_, D], F32)
                    b2_r = p6p.tile([128, D], F32)
                    nc.sync.dma_start(out=b2_r, in_=_rep_ap(t["b2d"], D))
                    for qt in range(QT_):
                        nc.vector.tensor_add(out=resb[:, qt, :],
                                             in0=res[:, qt, :],
                                             in1=b2_r)
                    for tq in range(QT_):
                        o2a = o2ps.tile([128, TQ], F32, tag="o2a")
                        o2b = o2ps.tile([128, TQ], F32, tag="o2b")
                        for dt_ in range(DFF // 128):
                            for hf, o2t in ((0, o2a), (1, o2b)):
                                nc.tensor.matmul(
                                    o2t,
                                    H1T[:, dt_, tq * 128:(tq + 1) * 128],
                                    w2s[:, dt_, hf * 512:(hf + 1) * 512],
                                    start=(dt_ == 0),
                                    stop=(dt_ == DFF // 128 - 1),
                                    skip_group_check=True)
                        fin = p6d.tile([128, D], F32, tag="fin")
                        for hf, o2t in ((0, o2a), (1, o2b)):
                            nc.vector.tensor_add(
                                out=fin[:, hf * 512:(hf + 1) * 512],
                                in0=o2t,
                                in1=resb[:, tq, hf * 512:(hf + 1) * 512])
                        stats = p6d.tile([128, 2, 6], F32, tag="stats2")
                        nc.vector.bn_stats(out=stats[:, 0, :],
                                           in_=fin[:, 0:512])
                        nc.vector.bn_stats(out=stats[:, 1, :],
                                           in_=fin[:, 512:1024])
                        mv = p6d.tile([128, 2], F32, tag="mv2")
                        nc.vector.bn_aggr(out=mv, in_=stats)
                        rstd = p6d.tile([128, 1], F32, tag="rstd2")
                        nc.scalar.activation(out=rstd, in_=mv[:, 1:2],
                                             func=AF.Sqrt, bias=eps_t)
                        nc.vector.reciprocal(out=rstd, in_=rstd)
                        nc.vector.tensor_scalar(
                            out=fin, in0=fin,
                            scalar1=mv[:, 0:1], scalar2=rstd,
                            op0=ALU.subtract, op1=ALU.mult)
                        nc.vector.tensor_mul(out=fin, in0=fin, in1=g2r)
                        nc.vector.tensor_add(out=fin, in0=fin, in1=be2r)
                        nc.sync.dma_start(
                            out=t["out"][tq * 128:(tq + 1) * 128, :],
                            in_=fin)


def build(repeat=1):
    nc = bacc.Bacc()

    t = {}
    t["xqT16"] = nc.dram_tensor("xqT16", [D, TQ], BF16, kind="ExternalInput")
    t["xT16"] = nc.dram_tensor("xT16", [D, S], BF16, kind="ExternalInput")
    t["xqf"] = nc.dram_tensor("xqf", [TQ, D], F32, kind="ExternalInput")
    t["wq16"] = nc.dram_tensor("wq16", [D, D], BF16, kind="ExternalInput")
    t["wk16"] = nc.dram_tensor("wk16", [D, D], BF16, kind="ExternalInput")
    t["wv16"] = nc.dram_tensor("wv16", [D, D], BF16, kind="ExternalInput")
    t["w1"] = nc.dram_tensor("w1", [D, DFF], BF16, kind="ExternalInput")
    t["w2"] = nc.dram_tensor("w2", [DFF, D], BF16, kind="ExternalInput")
    for name, dim in (("bq", D), ("bk", D), ("bv", D), ("b1d", DFF),
                      ("b2d", D), ("g1d", D), ("be1d", D), ("g2d", D),
                      ("be2d", D)):
        t[name] = nc.dram_tensor(name, [dim], F32, kind="ExternalInput")
    t["id16d"] = nc.dram_tensor("id16d", [128, 128], BF16,
                                kind="ExternalInput")
    t["idr32d"] = nc.dram_tensor("idr32d", [128, 128], F32,
                                 kind="ExternalInput")
    t["out"] = nc.dram_tensor("out", [TQ, D], F32, kind="ExternalOutput")

    if repeat == 1:
        reps, unroll = 1, 1
    else:
        assert repeat % BODY_REP == 0, (repeat, BODY_REP)
        reps, unroll = repeat // BODY_REP, BODY_REP

    with tile.TileContext(nc) as tc:
        stag = STAGGERED and reps > 1 and unroll == 4
        with (tc.For_i(0, reps, staggered_reset=stag)
              if reps > 1 else contextlib.nullcontext()):
            for r in range(unroll):
                if stag and r > 0:
                    tc.stage_boundary()
                _emit_body(nc, tc, t, r)
    nc.compile()
    return nc


_NC_CACHE = {}


def _get_nc(repeat=1):
    key = (USE_GELU, PHASES, BODY_REP, STAGGERED, repeat)
    if key not in _NC_CACHE:
        _NC_CACHE[key] = build(repeat)
    return _NC_CACHE[key]


def make_in_maps(x, Wq, bq, Wk, bk, Wv, bv, W1, b1, W2, b2, g1, be1, g2, be2):
    bf = ml_dtypes.bfloat16
    shared = {
        "wq16": np.ascontiguousarray(Wq.astype(bf)),
        "wk16": np.ascontiguousarray(Wk.astype(bf)),
        "wv16": np.ascontiguousarray(Wv.astype(bf)),
        "w1": np.ascontiguousarray(np.asarray(W1, np.float32).astype(bf)),
        "w2": np.ascontiguousarray(np.asarray(W2, np.float32).astype(bf)),
        "bq": np.asarray(bq, np.float32), "bk": np.asarray(bk, np.float32),
        "bv": np.asarray(bv, np.float32), "b1d": np.asarray(b1, np.float32),
        "b2d": np.asarray(b2, np.float32), "g1d": np.asarray(g1, np.float32),
        "be1d": np.asarray(be1, np.float32), "g2d": np.asarray(g2, np.float32),
        "be2d": np.asarray(be2, np.float32),
        "id16d": np.eye(128, dtype=bf),
        "idr32d": np.eye(128, dtype=np.float32),
    }
    in_maps = []
    xb_T16 = [np.ascontiguousarray(np.asarray(x[b], np.float32).T.astype(bf))
              for b in range(B)]
    for c in range(NCORES):
        b, chunk = divmod(c, 4)
        qoff = chunk * TQ
        xb = np.asarray(x[b], np.float32)
        m = dict(shared)
        m["xT16"] = xb_T16[b]
        m["xqT16"] = np.ascontiguousarray(xb_T16[b][:, qoff:qoff + TQ])
        m["xqf"] = np.ascontiguousarray(xb[qoff:qoff + TQ])
        in_maps.append(m)
    return in_maps


def kernel(x, Wq, bq, Wk, bk, Wv, bv, W1, b1, W2, b2, g1, be1, g2, be2):
    nc = _get_nc()
    in_maps = make_in_maps(x, Wq, bq, Wk, bk, Wv, bv, W1, b1, W2, b2,
                           g1, be1, g2, be2)
    try:
        r = run_bass_kernel_spmd(nc, in_maps, list(range(NCORES)))
    except Exception:
        # transient device errors (e.g. a wedged NeuronCore) usually clear
        # on retry
        import time as _time
        _time.sleep(2)
        r = run_bass_kernel_spmd(nc, in_maps, list(range(NCORES)))
    final = np.empty((B, S, D), np.float32)
    for c in range(NCORES):
        b, chunk = divmod(c, 4)
        qoff = chunk * TQ
        final[b, qoff:qoff + TQ] = r.results[c]["out"]
    return final
